# revision 10
# baseline (speedup 1.0000x reference)
"""Trainium2 Bass kernel for MultiHeadPosAttn (attention + BN + FFN + BN).

Sharding: data-parallel over batch across 8 NeuronCores (2 images/core).
BatchNorm batch statistics are combined with a tiny (2KB) AllReduce.

Math notes (verified exactly equivalent to the reference):
  - bk cancels in softmax (adds a per-query constant to every logit row).
  - bv cancels in BN1 (per-channel constant shift; softmax rows sum to 1).
  - b2 cancels in BN2 (per-channel constant shift).
  - PReLU(y) = Lrelu(y) with alpha = a (ACT supports a slope parameter).
  - softmax needs no max-subtraction: |logits| <= ~66 so exp() stays in
    fp32 range (max ~3e28 << 3.4e38).
Softmax denominator comes from an extra all-ones column in each head's
V^T block, so the attention matmul also produces sum_k(P) per query.
The V^T block for head h occupies lhsT columns so that the head's
output lands directly on its target partitions (even heads: d at
cols 0-63 + ones at col 64; odd heads: ones at col 63 + d at cols
64-127), avoiding any partition-shifting DMA.  The denominator row is
broadcast across partitions with a GPSIMD partition_broadcast (no DRAM
round trip).
"""

import numpy as np

import concourse.bass as bass
import concourse.bacc as bacc
import concourse.tile as tile
from concourse import mybir
from concourse import bass_utils

F32 = mybir.dt.float32
BF16 = mybir.dt.bfloat16
F16 = mybir.dt.float16

B, C, HH, WW = 16, 256, 32, 32
N = HH * WW              # 1024 spatial positions
NH, DH = 4, 64           # heads, head dim
DFF = 4 * C              # 1024
EPS = 1e-5
NCORES = 8
BL = B // NCORES         # 2 images per core
NCH = C // 128           # 2 channel chunks of 128
NFC = DFF // 128         # 8 ffn chunks
NNC = N // 128           # 8 position chunks


def _build(a_slope: float):
    nc = bacc.Bacc("TRN2", target_bir_lowering=False, debug=False,
                   num_devices=NCORES)

    x_d = nc.dram_tensor("x", [BL, C, N], F16, kind="ExternalInput")
    wq_d = nc.dram_tensor("wqT", [C, C], F16, kind="ExternalInput")
    wk_d = nc.dram_tensor("wkT", [C, C], F16, kind="ExternalInput")
    wv_d = nc.dram_tensor("wvT", [C, NH * DH], F16, kind="ExternalInput")
    bq_d = nc.dram_tensor("bq", [C], F32, kind="ExternalInput")
    w1_d = nc.dram_tensor("w1T", [C, DFF], F16, kind="ExternalInput")
    b1_d = nc.dram_tensor("b1s", [DFF], F32, kind="ExternalInput")
    w2_d = nc.dram_tensor("w2T", [DFF, C], F16, kind="ExternalInput")
    gam_d = nc.dram_tensor("gamma", [C], F32, kind="ExternalInput")
    bet_d = nc.dram_tensor("beta", [C], F32, kind="ExternalInput")
    out_d = nc.dram_tensor("out", [BL, C, N], F32, kind="ExternalOutput")

    with tile.TileContext(nc) as tc:
        _emit(tc, a_slope,
              x_d=x_d, wq_d=wq_d, wk_d=wk_d, wv_d=wv_d, bq_d=bq_d,
              w1_d=w1_d, b1_d=b1_d, w2_d=w2_d, gam_d=gam_d,
              bet_d=bet_d, out_d=out_d)
    nc.compile()
    return nc


def _emit(tc, a_slope, *, x_d, wq_d, wk_d, wv_d, bq_d, w1_d, b1_d, w2_d,
          gam_d, bet_d, out_d):
    nc = tc.nc
    from contextlib import ExitStack

    ctx = ExitStack()
    with ctx:
        const = ctx.enter_context(tc.tile_pool(name="const", bufs=1))
        data = ctx.enter_context(tc.tile_pool(name="data", bufs=1))
        work = ctx.enter_context(tc.tile_pool(name="work", bufs=1))
        dram = ctx.enter_context(tc.tile_pool(name="dram", bufs=1, space="DRAM"))

        # ---- loads, spread across engines so the QKV-critical tensors
        # (wq, x0, wk, wv) land ASAP; FFN weights queue behind ----
        xs = []
        for img in range(BL):
            xs.append(data.tile([128, NCH, N], F16, name=f"xs{img}",
                                tag=f"xs{img}"))
        wq_sb = const.tile([128, NCH, C], F16, name="wq_sb")
        wk_sb = const.tile([128, NCH, C], F16, name="wk_sb")
        wv_sb = const.tile([128, NCH, NH * DH], F16, name="wv_sb")
        w1_sb = const.tile([128, NCH, DFF], F16, name="w1_sb")
        w2_sb = const.tile([128, NFC, C], F16, name="w2_sb")

        wqr = wq_d.ap().rearrange("(k p) m -> p k m", p=128)
        wkr = wk_d.ap().rearrange("(k p) m -> p k m", p=128)
        x0r = x_d.ap()[0].rearrange("(c p) n -> p c n", p=128)
        x1r = x_d.ap()[1].rearrange("(c p) n -> p c n", p=128)
        # scalar engine: wq then x0 (first Q matmul needs both)
        nc.scalar.dma_start(out=wq_sb, in_=wqr)
        for ch in range(NCH):
            nc.scalar.dma_start(out=xs[0][:, ch, :], in_=x0r[:, ch, :])
        # sync engine: wk, x1, wv, then the small vectors
        nc.sync.dma_start(out=wk_sb, in_=wkr)
        for ch in range(NCH):
            nc.sync.dma_start(out=xs[1][:, ch, :], in_=x1r[:, ch, :])
        nc.sync.dma_start(out=wv_sb,
                          in_=wv_d.ap().rearrange("(k p) m -> p k m", p=128))
        # gpsimd: FFN weights (not needed until after attention)
        nc.gpsimd.dma_start(out=w1_sb,
                            in_=w1_d.ap().rearrange("(k p) m -> p k m", p=128))
        nc.gpsimd.dma_start(out=w2_sb,
                            in_=w2_d.ap().rearrange("(k p) m -> p k m", p=128))
        bq_sb = const.tile([128, NCH], F32, name="bq_sb")
        nc.sync.dma_start(out=bq_sb, in_=bq_d.ap().rearrange("(k p) -> p k", p=128))
        b1_sb = const.tile([128, NFC], F32, name="b1_sb")
        nc.sync.dma_start(out=b1_sb, in_=b1_d.ap().rearrange("(k p) -> p k", p=128))
        gam_sb = const.tile([128, NCH], F32, name="gam_sb")
        nc.sync.dma_start(out=gam_sb, in_=gam_d.ap().rearrange("(k p) -> p k", p=128))
        bet_sb = const.tile([128, NCH], F32, name="bet_sb")
        nc.sync.dma_start(out=bet_sb, in_=bet_d.ap().rearrange("(k p) -> p k", p=128))

        # warm-up collectives (prime the CC rings so BN1/BN2 are fast)
        warm_sb = const.tile([1, 64], F32, name="warm_sb")
        nc.vector.memset(warm_sb, 0.0)
        for wi in range(2):
            w_in = dram.tile([64], F32, name=f"warm{wi}_in", tag=f"warm{wi}_in")
            w_out = dram.tile([64], F32, name=f"warm{wi}_out",
                              tag=f"warm{wi}_out", addr_space="Shared")
            nc.sync.dma_start(out=w_in.unsqueeze(0), in_=warm_sb)
            nc.gpsimd.collective_compute(
                "AllReduce", mybir.AluOpType.add,
                replica_groups=[list(range(NCORES))],
                ins=[w_in.opt()], outs=[w_out.opt()])

        # ---- persistent SBUF tensors ----
        q_sb, k_sb, vt_sb, o_sb, mh_sb, u_sb = [], [], [], [], [], []
        for img in range(BL):
            q_sb.append(data.tile([128, NCH, N], F16, name=f"q{img}", tag=f"q{img}"))
            k_sb.append(data.tile([128, NCH, N], F16, name=f"k{img}", tag=f"k{img}"))
            vt_sb.append(data.tile([128, NNC, NH * 128], BF16, name=f"vt{img}",
                                   tag=f"vt{img}"))
            o_sb.append(data.tile([128, NCH, N], F32, name=f"o{img}", tag=f"o{img}"))
            mh_sb.append(data.tile([128, NCH, N], F16, name=f"mh{img}",
                                   tag=f"mh{img}"))
            u_sb.append(data.tile([128, NCH, N], F32, name=f"u{img}", tag=f"u{img}"))

        # V^T layout per head block (128 cols): even heads [v(64) | 1 | 0*63],
        # odd heads [1 | 0*63 | v(64)] -- the ones (denominator) column must
        # land on a 32-aligned PSUM partition (0 or 64).
        for img in range(BL):
            vt4 = vt_sb[img].rearrange("p a (h d) -> p a h d", d=128)
            for h in range(NH):
                if h % 2 == 0:
                    nc.vector.memset(vt4[:, :, h, DH + 1:128], 0.0)
                    nc.vector.memset(vt4[:, :, h, DH:DH + 1], 1.0)
                else:
                    nc.vector.memset(vt4[:, :, h, 1:DH], 0.0)
                    nc.vector.memset(vt4[:, :, h, 0:1], 1.0)

        st1 = work.tile([128, NCH, BL * 2, 6], F32, name="bn1_stats",
                        tag="bn1_stats")
        st2 = work.tile([128, NCH, BL * 2, 6], F32, name="bn2_stats",
                        tag="bn2_stats")

        # =========== QKV + attention ===========
        # emit img0's QKV, then img0's heads with img1's QKV interleaved
        # into the PE slack (attention is ACT/exp-bound), then img1's heads.
        with tc.tile_pool(name="etps", bufs=2, space="PSUM") as etps, \
             tc.tile_pool(name="oaps", bufs=2, space="PSUM") as oaps:

            def emit_qkv_q(img):
                for mc in range(NCH):
                    qp = etps.tile([128, N], F32, tag="et", bufs=2)
                    for kc in range(NCH):
                        for mv in range(2):
                            nc.tensor.matmul(
                                qp[:, mv * 512:(mv + 1) * 512],
                                lhsT=(wq_sb[:, kc, mc * 128:(mc + 1) * 128]),
                                rhs=(xs[img][:, kc, mv * 512:(mv + 1) * 512]),
                                start=(kc == 0), stop=(kc == NCH - 1))
                    nc.vector.tensor_scalar_add(q_sb[img][:, mc, :], qp,
                                                bq_sb[:, mc:mc + 1])

            def emit_qkv_k(img):
                for mc in range(NCH):
                    kp = etps.tile([128, N], F32, tag="et", bufs=2)
                    for kc in range(NCH):
                        for mv in range(2):
                            nc.tensor.matmul(
                                kp[:, mv * 512:(mv + 1) * 512],
                                lhsT=(wk_sb[:, kc, mc * 128:(mc + 1) * 128]),
                                rhs=(xs[img][:, kc, mv * 512:(mv + 1) * 512]),
                                start=(kc == 0), stop=(kc == NCH - 1))
                    nc.vector.tensor_copy(k_sb[img][:, mc, :], kp)

            def emit_qkv_v(img, pcs):
                vt4 = vt_sb[img].rearrange("p a (h d) -> p a h d", d=128)
                for pc in pcs:
                    vp = etps.tile([128, N], F32, tag="et", bufs=2)
                    for kc in range(NCH):
                        nc.tensor.matmul(
                            vp[:, 0:NH * DH],
                            lhsT=(xs[img][:, kc, pc * 128:(pc + 1) * 128]),
                            rhs=(wv_sb[:, kc, :]),
                            start=(kc == 0), stop=(kc == NCH - 1))
                    for h in range(NH):
                        dst0 = 0 if h % 2 == 0 else 64
                        nc.vector.tensor_copy(
                            vt4[:, pc, h, dst0:dst0 + DH],
                            vp[:, h * DH:(h + 1) * DH])

            def emit_head(img, h):
                hc, ho = h // 2, (h % 2) * 64
                denp = 64 if h % 2 == 0 else 0
                q_h = q_sb[img][ho:ho + 64, hc, :]
                k_h = k_sb[img][ho:ho + 64, hc, :]
                oaug = oaps.tile([128, N], F32, tag="oaug", bufs=2)
                for pc in range(NNC):
                    et = etps.tile([128, N], F32, tag="et", bufs=2)
                    for mv in range(2):
                        nc.tensor.matmul(
                            et[:, mv * 512:(mv + 1) * 512],
                            lhsT=(k_h[:, pc * 128:(pc + 1) * 128]),
                            rhs=(q_h[:, mv * 512:(mv + 1) * 512]),
                            start=True, stop=True)
                    p_t = work.tile([128, N], BF16, name="p_t", tag="p_t", bufs=6)
                    nc.scalar.activation(p_t, et,
                                         mybir.ActivationFunctionType.Exp)
                    for mv in range(2):
                        nc.tensor.matmul(
                            oaug[:, mv * 512:(mv + 1) * 512],
                            lhsT=(vt_sb[img][:, pc, h * 128:(h + 1) * 128]),
                            rhs=(p_t[:, mv * 512:(mv + 1) * 512]),
                            start=(pc == 0), stop=(pc == NNC - 1))
                # softmax denominator: row `denp` of oaug.  Copy to SBUF,
                # broadcast across all partitions on GPSIMD, reciprocal
                # (base-0 custom DVE op), then scale the head's 64 rows.
                dsb = work.tile([128, N], F32, name="dsb", tag="dsb", bufs=2)
                nc.vector.tensor_copy(dsb[denp:denp + 1, :],
                                      oaug[denp:denp + 1, :])
                if denp != 0:
                    # partition_broadcast reads ABSOLUTE partition 0 on HW:
                    # bounce the row down with a tiny SBUF->SBUF DMA first.
                    dsb0 = work.tile([128, N], F32, name="dsb0", tag="dsb0",
                                     bufs=2)
                    nc.sync.dma_start(out=dsb0[0:1, :], in_=dsb[denp:denp + 1, :])
                    dsb = dsb0
                dbc = work.tile([128, N], F32, name="dbc", tag="dbc", bufs=2)
                nc.gpsimd.partition_broadcast(dbc, dsb[0:1, :])
                rbc = work.tile([128, N], F32, name="rbc", tag="rbc", bufs=2)
                nc.vector.reciprocal_approx_fast(out=rbc, in_=dbc)
                nc.vector.tensor_mul(o_sb[img][ho:ho + 64, hc, :],
                                     oaug[ho:ho + 64, :], rbc[ho:ho + 64, :])
                if h % 2 == 1:
                    # both heads of chunk hc done -> residual + local stats
                    nc.vector.tensor_add(o_sb[img][:, hc, :],
                                         o_sb[img][:, hc, :],
                                         xs[img][:, hc, :])
                    for sg in range(2):
                        nc.vector.bn_stats(
                            out=st1[:, hc, img * 2 + sg, :],
                            in_=o_sb[img][:, hc, sg * 512:(sg + 1) * 512])

            emit_qkv_q(0)
            emit_qkv_k(0)
            emit_qkv_v(0, range(NNC))
            emit_head(0, 0)
            emit_qkv_q(1)
            emit_head(0, 1)
            emit_qkv_k(1)
            emit_head(0, 2)
            emit_qkv_v(1, range(0, 4))
            emit_head(0, 3)
            emit_qkv_v(1, range(4, NNC))
            for h in range(NH):
                emit_head(1, h)

        # =========== BN1 ===========
        s1_sb = work.tile([128, NCH], F32, name="s1_sb", tag="bns")
        t1_sb = work.tile([128, NCH], F32, name="t1_sb", tag="bnt")
        eps_unused = None
        cc1 = _bn_allreduce(tc, nc, work, dram, "bn1", st1)
        _bn_finish(tc, nc, work, "bn1", cc_out=cc1, gam_sb=gam_sb,
                   bet_sb=bet_sb, scale_out=s1_sb, shift_out=t1_sb)
        # apply: mh = s*(o+x) + t ; split across ACT (ch0) and DVE (ch1)
        for img in range(BL):
            nc.scalar.activation(mh_sb[img][:, 0, :], o_sb[img][:, 0, :],
                                 mybir.ActivationFunctionType.Identity,
                                 bias=t1_sb[:, 0:1], scale=s1_sb[:, 0:1])
            nc.vector.tensor_scalar(
                out=mh_sb[img][:, 1, :], in0=o_sb[img][:, 1, :],
                scalar1=s1_sb[:, 1:2], scalar2=t1_sb[:, 1:2],
                op0=mybir.AluOpType.mult, op1=mybir.AluOpType.add)

        # =========== FFN (mc-major W2 so stats start early) ===========
        ffs = [[work.tile([128, N], F16, name=f"ffs{img}_{fc}",
                          tag=f"ffs{fc}", bufs=2) for fc in range(NFC)]
               for img in range(BL)]
        with tc.tile_pool(name="ffps", bufs=2, space="PSUM") as ffps, \
             tc.tile_pool(name="ops2", bufs=2, space="PSUM") as ops2:
            for img in range(BL):
                for mc in range(NCH):
                    outp = ops2.tile([128, N], F32, tag="outp", bufs=2)
                    for fc in range(NFC):
                        if mc == 0:
                            fp = ffps.tile([128, N], F32, tag="fp", bufs=2)
                            for kc in range(NCH):
                                for mv in range(2):
                                    nc.tensor.matmul(
                                        fp[:, mv * 512:(mv + 1) * 512],
                                        lhsT=(w1_sb[:, kc, fc * 128:(fc + 1) * 128]),
                                        rhs=(mh_sb[img][:, kc, mv * 512:(mv + 1) * 512]),
                                        start=(kc == 0), stop=(kc == NCH - 1))
                            # PReLU(y) via ACT Prelu (alpha = slope; Lrelu ignores alpha on this HW)
                            nc.scalar.activation(
                                ffs[img][fc], fp,
                                mybir.ActivationFunctionType.Prelu,
                                bias=b1_sb[:, fc:fc + 1], alpha=a_slope)
                        for mv in range(2):
                            nc.tensor.matmul(
                                outp[:, mv * 512:(mv + 1) * 512],
                                lhsT=(w2_sb[:, fc, mc * 128:(mc + 1) * 128]),
                                rhs=(ffs[img][fc][:, mv * 512:(mv + 1) * 512]),
                                start=(fc == 0), stop=(fc == NFC - 1))
                    nc.vector.tensor_add(u_sb[img][:, mc, :], outp,
                                         mh_sb[img][:, mc, :])
                    for sg in range(2):
                        nc.vector.bn_stats(
                            out=st2[:, mc, img * 2 + sg, :],
                            in_=u_sb[img][:, mc, sg * 512:(sg + 1) * 512])

        # =========== BN2 + output ===========
        s2_sb = work.tile([128, NCH], F32, name="s2_sb", tag="bns2")
        t2_sb = work.tile([128, NCH], F32, name="t2_sb", tag="bnt2")
        cc2 = _bn_allreduce(tc, nc, work, dram, "bn2", st2)
        _bn_finish(tc, nc, work, "bn2", cc_out=cc2, gam_sb=gam_sb,
                   bet_sb=bet_sb, scale_out=s2_sb, shift_out=t2_sb)
        for img in range(BL):
            outr = out_d.ap()[img].rearrange("(c p) n -> p c n", p=128)
            # ch0 apply on ACT, ch1 on DVE (o_sb is dead -> reuse as staging)
            nc.scalar.activation(o_sb[img][:, 0, :], u_sb[img][:, 0, :],
                                 mybir.ActivationFunctionType.Identity,
                                 bias=t2_sb[:, 0:1], scale=s2_sb[:, 0:1])
            nc.sync.dma_start(out=outr[:, 0, :], in_=o_sb[img][:, 0, :])
            nc.vector.tensor_scalar(
                out=o_sb[img][:, 1, :], in0=u_sb[img][:, 1, :],
                scalar1=s2_sb[:, 1:2], scalar2=t2_sb[:, 1:2],
                op0=mybir.AluOpType.mult, op1=mybir.AluOpType.add)
            nc.gpsimd.dma_start(out=outr[:, 1, :], in_=o_sb[img][:, 1, :])


def _bn_allreduce(tc, nc, work, dram, name, stats):
    """Aggregate all local bn_stats, pack [mean, msq], one 2KB AllReduce."""
    mv_t = work.tile([128, NCH, 2], F32, name=f"{name}_mv", tag=f"{name}_mv")
    pk = work.tile([128, NCH, 2], F32, name=f"{name}_pk", tag=f"{name}_pk")
    for ch in range(NCH):
        nc.vector.bn_aggr(out=mv_t[:, ch, :], in_=stats[:, ch, :, :])
        nc.vector.tensor_mul(pk[:, ch, 0:1], mv_t[:, ch, 0:1], mv_t[:, ch, 0:1])
        nc.vector.tensor_add(pk[:, ch, 1:2], mv_t[:, ch, 1:2], pk[:, ch, 0:1])
        nc.vector.tensor_copy(pk[:, ch, 0:1], mv_t[:, ch, 0:1])
    cc_in = dram.tile([128 * NCH * 2], F32, name=f"{name}_cc_in",
                      tag=f"{name}_cc_in")
    cc_out = dram.tile([128 * NCH * 2], F32, name=f"{name}_cc_out",
                       tag=f"{name}_cc_out", addr_space="Shared")
    nc.sync.dma_start(out=cc_in.rearrange("(p k) -> p k", p=128), in_=pk)
    nc.gpsimd.collective_compute(
        "AllReduce", mybir.AluOpType.add,
        replica_groups=[list(range(NCORES))],
        ins=[cc_in.opt()], outs=[cc_out.opt()])
    return cc_out


def _bn_finish(tc, nc, work, name, *, cc_out, gam_sb, bet_sb,
               scale_out, shift_out):
    """Turn the AllReduced [mean, msq] sums into per-channel scale/shift.
    rsqrt is DVE-only (bit-trick seed + Newton) to avoid an ACT table
    switch on the critical path."""
    sg_t = work.tile([128, NCH, 2], F32, name=f"{name}_sg", tag=f"{name}_sg")
    nc.sync.dma_start(out=sg_t, in_=cc_out.rearrange("(p k) -> p k", p=128))
    g8 = work.tile([128, NCH, 2], F32, name=f"{name}_g8", tag=f"{name}_g8")
    nc.vector.tensor_scalar_mul(g8, sg_t, 1.0 / NCORES)
    # var = msq - mean^2 + eps   (both channel chunks at once)
    var_t = work.tile([128, NCH], F32, name=f"{name}_var", tag=f"{name}_var")
    nc.vector.tensor_mul(var_t, g8[:, :, 0], g8[:, :, 0])
    nc.vector.tensor_sub(var_t, g8[:, :, 1], var_t)
    nc.vector.tensor_scalar_add(var_t, var_t, EPS)
    # rstd = rsqrt(var): bit-trick seed + 2 Newton iterations (~fp32)
    rs = work.tile([128, NCH], F32, name=f"{name}_rs", tag=f"{name}_rs")
    vi = var_t.bitcast(mybir.dt.int32)
    ri = rs.bitcast(mybir.dt.int32)
    nc.vector.tensor_scalar(out=ri, in0=vi, scalar1=1, scalar2=None,
                            op0=mybir.AluOpType.arith_shift_right)
    nc.vector.tensor_scalar(out=ri, in0=ri, scalar1=-1, scalar2=0x5f3759df,
                            op0=mybir.AluOpType.mult,
                            op1=mybir.AluOpType.add)
    half = work.tile([128, NCH], F32, name=f"{name}_half", tag=f"{name}_half")
    nc.vector.tensor_scalar_mul(half, var_t, -0.5)
    tmp = work.tile([128, NCH], F32, name=f"{name}_tmp", tag=f"{name}_tmp")
    for _ in range(2):
        nc.vector.tensor_mul(tmp, rs, rs)
        nc.vector.tensor_mul(tmp, tmp, half)
        nc.vector.tensor_scalar_add(tmp, tmp, 1.5)
        nc.vector.tensor_mul(rs, rs, tmp)
    nc.vector.tensor_mul(scale_out, gam_sb, rs)
    # shift = beta - mean * scale
    nc.vector.tensor_mul(tmp, g8[:, :, 0], scale_out)
    nc.vector.tensor_sub(shift_out, bet_sb, tmp)


_COMPILED = None


def _get_compiled(a_slope: float):
    global _COMPILED
    if _COMPILED is None or _COMPILED[0] != a_slope:
        _COMPILED = (a_slope, _build(a_slope))
    return _COMPILED[1]


def _prep_inputs(inputs):
    x = np.ascontiguousarray(np.asarray(inputs["x"], dtype=np.float32))
    Wq = np.asarray(inputs["Wq"], dtype=np.float32)
    Wk = np.asarray(inputs["Wk"], dtype=np.float32)
    Wv = np.asarray(inputs["Wv"], dtype=np.float32)
    bq = np.asarray(inputs["bq"], dtype=np.float32)
    W1 = np.asarray(inputs["W1"], dtype=np.float32)
    b1 = np.asarray(inputs["b1"], dtype=np.float32)
    W2 = np.asarray(inputs["W2"], dtype=np.float32)
    gamma = np.asarray(inputs["gamma"], dtype=np.float32)
    beta = np.asarray(inputs["beta"], dtype=np.float32)

    wqT = np.ascontiguousarray(Wq.reshape(C, C).T.astype(np.float16))
    wkT = np.ascontiguousarray(Wk.reshape(C, C).T.astype(np.float16))
    wvT = np.zeros((C, NH * DH), dtype=np.float16)
    for h in range(NH):
        wvT[:, h * DH:(h + 1) * DH] = Wv[h].T.astype(np.float16)
    common = {
        "wqT": wqT, "wkT": wkT, "wvT": wvT,
        "bq": np.ascontiguousarray(bq.reshape(C)),
        "w1T": np.ascontiguousarray(W1.T.astype(np.float16)),
        "b1s": np.ascontiguousarray(b1),
        "w2T": np.ascontiguousarray(W2.T.astype(np.float16)),
        "gamma": np.ascontiguousarray(gamma),
        "beta": np.ascontiguousarray(beta),
    }
    x16 = x.astype(np.float16)
    in_maps = []
    for c in range(NCORES):
        m = dict(common)
        m["x"] = np.ascontiguousarray(
            x16[c * BL:(c + 1) * BL].reshape(BL, C, N))
        in_maps.append(m)
    return in_maps


def kernel_ex(trace=False, **inputs):
    a_slope = float(np.asarray(inputs["a"]))
    nc = _get_compiled(a_slope)
    in_maps = _prep_inputs(inputs)
    res = bass_utils.run_bass_kernel_spmd(
        nc, in_maps, core_ids=list(range(NCORES)), trace=trace)
    out = np.empty((B, C, N), dtype=np.float32)
    for c in range(NCORES):
        out[c * BL:(c + 1) * BL] = res.results[c]["out"]
    return out.reshape(B, C, HH, WW), res


def kernel(**inputs):
    out, _ = kernel_ex(False, **inputs)
    return out


# revision 11
# speedup vs baseline: 1.2726x; 1.2726x over previous
"""Trainium2 Bass kernel for MultiHeadPosAttn (attention + BN + FFN + BN).

Sharding: data-parallel over batch across 8 NeuronCores (2 images/core).
BatchNorm batch statistics are combined with a tiny (2KB) AllReduce.

Math notes (verified exactly equivalent to the reference):
  - bk cancels in softmax (adds a per-query constant to every logit row).
  - bv cancels in BN1 (per-channel constant shift; softmax rows sum to 1).
  - b2 cancels in BN2 (per-channel constant shift).
  - PReLU(y) = Lrelu(y) with alpha = a (ACT supports a slope parameter).
  - softmax needs no max-subtraction: |logits| <= ~66 so exp() stays in
    fp32 range (max ~3e28 << 3.4e38).
Softmax denominator comes from an extra all-ones column in each head's
V^T block, so the attention matmul also produces sum_k(P) per query.
The V^T block for head h occupies lhsT columns so that the head's
output lands directly on its target partitions (even heads: d at
cols 0-63 + ones at col 64; odd heads: ones at col 63 + d at cols
64-127), avoiding any partition-shifting DMA.  The denominator row is
broadcast across partitions with a GPSIMD partition_broadcast (no DRAM
round trip).
"""

import numpy as np

import concourse.bass as bass
import concourse.bacc as bacc
import concourse.tile as tile
from concourse import mybir
from concourse import bass_utils

F32 = mybir.dt.float32
BF16 = mybir.dt.bfloat16
F16 = mybir.dt.float16

B, C, HH, WW = 16, 256, 32, 32
N = HH * WW              # 1024 spatial positions
NH, DH = 4, 64           # heads, head dim
DFF = 4 * C              # 1024
EPS = 1e-5
NCORES = 8
BL = B // NCORES         # 2 images per core
NCH = C // 128           # 2 channel chunks of 128
NFC = DFF // 128         # 8 ffn chunks
NNC = N // 128           # 8 position chunks


def _build(a_slope: float):
    nc = bacc.Bacc("TRN2", target_bir_lowering=False, debug=False,
                   num_devices=NCORES)

    x_d = nc.dram_tensor("x", [BL, C, N], F16, kind="ExternalInput")
    wq_d = nc.dram_tensor("wqT", [C, C], F16, kind="ExternalInput")
    wk_d = nc.dram_tensor("wkT", [C, C], F16, kind="ExternalInput")
    wv_d = nc.dram_tensor("wvT", [C, NH * DH], F16, kind="ExternalInput")
    bq_d = nc.dram_tensor("bq", [C], F32, kind="ExternalInput")
    w1_d = nc.dram_tensor("w1T", [C, DFF], F16, kind="ExternalInput")
    b1_d = nc.dram_tensor("b1s", [DFF], F32, kind="ExternalInput")
    w2_d = nc.dram_tensor("w2T", [DFF, C], F16, kind="ExternalInput")
    gam_d = nc.dram_tensor("gamma", [C], F32, kind="ExternalInput")
    bet_d = nc.dram_tensor("beta", [C], F32, kind="ExternalInput")
    out_d = nc.dram_tensor("out", [BL, C, N], F32, kind="ExternalOutput")

    with tile.TileContext(nc) as tc:
        _emit(tc, a_slope,
              x_d=x_d, wq_d=wq_d, wk_d=wk_d, wv_d=wv_d, bq_d=bq_d,
              w1_d=w1_d, b1_d=b1_d, w2_d=w2_d, gam_d=gam_d,
              bet_d=bet_d, out_d=out_d)
    nc.compile()
    return nc


def _emit(tc, a_slope, *, x_d, wq_d, wk_d, wv_d, bq_d, w1_d, b1_d, w2_d,
          gam_d, bet_d, out_d):
    nc = tc.nc
    from contextlib import ExitStack

    ctx = ExitStack()
    with ctx:
        const = ctx.enter_context(tc.tile_pool(name="const", bufs=1))
        data = ctx.enter_context(tc.tile_pool(name="data", bufs=1))
        work = ctx.enter_context(tc.tile_pool(name="work", bufs=1))
        dram = ctx.enter_context(tc.tile_pool(name="dram", bufs=1, space="DRAM"))

        # ---- loads, spread across engines so the QKV-critical tensors
        # (wq, x0, wk, wv) land ASAP; FFN weights queue behind ----
        xs = []
        for img in range(BL):
            xs.append(data.tile([128, NCH, N], F16, name=f"xs{img}",
                                tag=f"xs{img}"))
        wq_sb = const.tile([128, NCH, C], F16, name="wq_sb")
        wk_sb = const.tile([128, NCH, C], F16, name="wk_sb")
        wv_sb = const.tile([128, NCH, NH * DH], F16, name="wv_sb")
        w1_sb = const.tile([128, NCH, DFF], F16, name="w1_sb")
        w2_sb = const.tile([128, NFC, C], F16, name="w2_sb")

        wqr = wq_d.ap().rearrange("(k p) m -> p k m", p=128)
        wkr = wk_d.ap().rearrange("(k p) m -> p k m", p=128)
        x0r = x_d.ap()[0].rearrange("(c p) n -> p c n", p=128)
        x1r = x_d.ap()[1].rearrange("(c p) n -> p c n", p=128)
        # scalar engine: wq then x0 (first Q matmul needs both)
        nc.scalar.dma_start(out=wq_sb, in_=wqr)
        for ch in range(NCH):
            nc.scalar.dma_start(out=xs[0][:, ch, :], in_=x0r[:, ch, :])
        # sync engine: wk, x1, wv, then the small vectors
        nc.sync.dma_start(out=wk_sb, in_=wkr)
        for ch in range(NCH):
            nc.sync.dma_start(out=xs[1][:, ch, :], in_=x1r[:, ch, :])
        nc.sync.dma_start(out=wv_sb,
                          in_=wv_d.ap().rearrange("(k p) m -> p k m", p=128))
        # gpsimd: FFN weights (not needed until after attention)
        nc.gpsimd.dma_start(out=w1_sb,
                            in_=w1_d.ap().rearrange("(k p) m -> p k m", p=128))
        nc.gpsimd.dma_start(out=w2_sb,
                            in_=w2_d.ap().rearrange("(k p) m -> p k m", p=128))
        bq_sb = const.tile([128, NCH], F32, name="bq_sb")
        nc.sync.dma_start(out=bq_sb, in_=bq_d.ap().rearrange("(k p) -> p k", p=128))
        b1_sb = const.tile([128, NFC], F32, name="b1_sb")
        nc.sync.dma_start(out=b1_sb, in_=b1_d.ap().rearrange("(k p) -> p k", p=128))
        gam_sb = const.tile([128, NCH], F32, name="gam_sb")
        nc.sync.dma_start(out=gam_sb, in_=gam_d.ap().rearrange("(k p) -> p k", p=128))
        bet_sb = const.tile([128, NCH], F32, name="bet_sb")
        nc.sync.dma_start(out=bet_sb, in_=bet_d.ap().rearrange("(k p) -> p k", p=128))

        # warm-up collectives (prime the CC rings so BN1/BN2 are fast)
        warm_sb = const.tile([1, 64], F32, name="warm_sb")
        nc.vector.memset(warm_sb, 0.0)
        for wi in range(2):
            w_in = dram.tile([64], F32, name=f"warm{wi}_in", tag=f"warm{wi}_in")
            w_out = dram.tile([64], F32, name=f"warm{wi}_out",
                              tag=f"warm{wi}_out", addr_space="Shared")
            nc.sync.dma_start(out=w_in.unsqueeze(0), in_=warm_sb)
            nc.gpsimd.collective_compute(
                "AllReduce", mybir.AluOpType.add,
                replica_groups=[list(range(NCORES))],
                ins=[w_in.opt()], outs=[w_out.opt()])

        # ---- persistent SBUF tensors ----
        q_sb, k_sb, vt_sb, o_sb, mh_sb, u_sb = [], [], [], [], [], []
        for img in range(BL):
            q_sb.append(data.tile([128, NCH, N], F16, name=f"q{img}", tag=f"q{img}"))
            k_sb.append(data.tile([128, NCH, N], F16, name=f"k{img}", tag=f"k{img}"))
            vt_sb.append(data.tile([128, NNC, NH * 128], BF16, name=f"vt{img}",
                                   tag=f"vt{img}"))
            o_sb.append(data.tile([128, NCH, N], F32, name=f"o{img}", tag=f"o{img}"))
            mh_sb.append(data.tile([128, NCH, N], F16, name=f"mh{img}",
                                   tag=f"mh{img}"))
            u_sb.append(data.tile([128, NCH, N], F32, name=f"u{img}", tag=f"u{img}"))

        # V^T layout per head block (128 cols): even heads [v(64) | 1 | 0*63],
        # odd heads [1 | 0*63 | v(64)] -- the ones (denominator) column must
        # land on a 32-aligned PSUM partition (0 or 64).
        for img in range(BL):
            vt4 = vt_sb[img].rearrange("p a (h d) -> p a h d", d=128)
            for h in range(NH):
                if h % 2 == 0:
                    nc.vector.memset(vt4[:, :, h, DH + 1:128], 0.0)
                    nc.vector.memset(vt4[:, :, h, DH:DH + 1], 1.0)
                else:
                    nc.vector.memset(vt4[:, :, h, 1:DH], 0.0)
                    nc.vector.memset(vt4[:, :, h, 0:1], 1.0)

        st1 = work.tile([128, NCH, BL * 2, 6], F32, name="bn1_stats",
                        tag="bn1_stats")
        st2 = work.tile([128, NCH, BL * 2, 6], F32, name="bn2_stats",
                        tag="bn2_stats")

        # =========== QKV phase (own PSUM pools, closed before attention) ====
        with tc.tile_pool(name="qkps", bufs=2, space="PSUM") as qkps, \
             tc.tile_pool(name="vtps", bufs=2, space="PSUM") as vtps:

            def emit_qkv_q(img):
                for mc in range(NCH):
                    qp = qkps.tile([128, N], F32, tag="qp", bufs=2)
                    for kc in range(NCH):
                        for mv in range(2):
                            nc.tensor.matmul(
                                qp[:, mv * 512:(mv + 1) * 512],
                                lhsT=(wq_sb[:, kc, mc * 128:(mc + 1) * 128]),
                                rhs=(xs[img][:, kc, mv * 512:(mv + 1) * 512]),
                                start=(kc == 0), stop=(kc == NCH - 1))
                    nc.vector.tensor_scalar_add(q_sb[img][:, mc, :], qp,
                                                bq_sb[:, mc:mc + 1])

            def emit_qkv_k(img):
                for mc in range(NCH):
                    kp = qkps.tile([128, N], F32, tag="qp", bufs=2)
                    for kc in range(NCH):
                        for mv in range(2):
                            nc.tensor.matmul(
                                kp[:, mv * 512:(mv + 1) * 512],
                                lhsT=(wk_sb[:, kc, mc * 128:(mc + 1) * 128]),
                                rhs=(xs[img][:, kc, mv * 512:(mv + 1) * 512]),
                                start=(kc == 0), stop=(kc == NCH - 1))
                    nc.vector.tensor_copy(k_sb[img][:, mc, :], kp)

            def emit_qkv_v(img, pcs):
                vt4 = vt_sb[img].rearrange("p a (h d) -> p a h d", d=128)
                for pc in pcs:
                    vp = vtps.tile([128, NH * DH], F32, tag="vp", bufs=2)
                    for kc in range(NCH):
                        nc.tensor.matmul(
                            vp,
                            lhsT=(xs[img][:, kc, pc * 128:(pc + 1) * 128]),
                            rhs=(wv_sb[:, kc, :]),
                            start=(kc == 0), stop=(kc == NCH - 1))
                    for h in range(NH):
                        dst0 = 0 if h % 2 == 0 else 64
                        nc.vector.tensor_copy(
                            vt4[:, pc, h, dst0:dst0 + DH],
                            vp[:, h * DH:(h + 1) * DH])

            def emit_head(img, h):
                hc, ho = h // 2, (h % 2) * 64
                denp = 64 if h % 2 == 0 else 0
                q_h = q_sb[img][ho:ho + 64, hc, :]
                k_h = k_sb[img][ho:ho + 64, hc, :]
                oaug = oaps.tile([128, N], F32, tag="oaug", bufs=2)
                for pc in range(NNC):
                    et = etps.tile([128, N], F32, tag="et", bufs=2)
                    for mv in range(2):
                        nc.tensor.matmul(
                            et[:, mv * 512:(mv + 1) * 512],
                            lhsT=(k_h[:, pc * 128:(pc + 1) * 128]),
                            rhs=(q_h[:, mv * 512:(mv + 1) * 512]),
                            start=True, stop=True)
                    p_t = work.tile([128, N], BF16, name="p_t", tag="p_t", bufs=6)
                    nc.scalar.activation(p_t, et,
                                         mybir.ActivationFunctionType.Exp)
                    for mv in range(2):
                        nc.tensor.matmul(
                            oaug[:, mv * 512:(mv + 1) * 512],
                            lhsT=(vt_sb[img][:, pc, h * 128:(h + 1) * 128]),
                            rhs=(p_t[:, mv * 512:(mv + 1) * 512]),
                            start=(pc == 0), stop=(pc == NNC - 1))
                # softmax denominator: row `denp` of oaug.  Copy to SBUF,
                # broadcast across all partitions on GPSIMD, reciprocal
                # (base-0 custom DVE op), then scale the head's 64 rows.
                dsb = work.tile([128, N], F32, name="dsb", tag="dsb", bufs=2)
                nc.vector.tensor_copy(dsb[denp:denp + 1, :],
                                      oaug[denp:denp + 1, :])
                if denp != 0:
                    # partition_broadcast reads ABSOLUTE partition 0 on HW:
                    # bounce the row down with a tiny SBUF->SBUF DMA first.
                    dsb0 = work.tile([128, N], F32, name="dsb0", tag="dsb0",
                                     bufs=2)
                    nc.gpsimd.dma_start(out=dsb0[0:1, :], in_=dsb[denp:denp + 1, :])
                    dsb = dsb0
                dbc = work.tile([128, N], F32, name="dbc", tag="dbc", bufs=2)
                nc.gpsimd.partition_broadcast(dbc, dsb[0:1, :])
                rbc = work.tile([128, N], F32, name="rbc", tag="rbc", bufs=2)
                nc.vector.reciprocal_approx_fast(out=rbc, in_=dbc)
                nc.vector.tensor_mul(o_sb[img][ho:ho + 64, hc, :],
                                     oaug[ho:ho + 64, :], rbc[ho:ho + 64, :])
                if h % 2 == 1:
                    # both heads of chunk hc done -> residual + local stats
                    nc.vector.tensor_add(o_sb[img][:, hc, :],
                                         o_sb[img][:, hc, :],
                                         xs[img][:, hc, :])
                    for sg in range(2):
                        nc.vector.bn_stats(
                            out=st1[:, hc, img * 2 + sg, :],
                            in_=o_sb[img][:, hc, sg * 512:(sg + 1) * 512])

            emit_qkv_q(0)
            emit_qkv_k(0)
            emit_qkv_v(0, range(NNC))
            emit_qkv_q(1)
            emit_qkv_k(1)
            emit_qkv_v(1, range(NNC))

        with tc.tile_pool(name="etps", bufs=2, space="PSUM") as etps, \
             tc.tile_pool(name="oaps", bufs=2, space="PSUM") as oaps:
            for img in range(BL):
                for h in range(NH):
                    emit_head(img, h)

        # =========== BN1 ===========
        s1_sb = work.tile([128, NCH], F32, name="s1_sb", tag="bns")
        t1_sb = work.tile([128, NCH], F32, name="t1_sb", tag="bnt")
        eps_unused = None
        cc1 = _bn_allreduce(tc, nc, work, dram, "bn1", st1)
        _bn_finish(tc, nc, work, "bn1", cc_out=cc1, gam_sb=gam_sb,
                   bet_sb=bet_sb, scale_out=s1_sb, shift_out=t1_sb)
        # apply: mh = s*(o+x) + t ; split across ACT (ch0) and DVE (ch1)
        for img in range(BL):
            nc.scalar.activation(mh_sb[img][:, 0, :], o_sb[img][:, 0, :],
                                 mybir.ActivationFunctionType.Identity,
                                 bias=t1_sb[:, 0:1], scale=s1_sb[:, 0:1])
            nc.vector.tensor_scalar(
                out=mh_sb[img][:, 1, :], in0=o_sb[img][:, 1, :],
                scalar1=s1_sb[:, 1:2], scalar2=t1_sb[:, 1:2],
                op0=mybir.AluOpType.mult, op1=mybir.AluOpType.add)

        # =========== FFN (mc-major W2 so stats start early) ===========
        ffs = [[work.tile([128, N], F16, name=f"ffs{img}_{fc}",
                          tag=f"ffs{fc}", bufs=2) for fc in range(NFC)]
               for img in range(BL)]
        with tc.tile_pool(name="ffps", bufs=2, space="PSUM") as ffps, \
             tc.tile_pool(name="ops2", bufs=2, space="PSUM") as ops2:
            for img in range(BL):
                for mc in range(NCH):
                    outp = ops2.tile([128, N], F32, tag="outp", bufs=2)
                    for fc in range(NFC):
                        if mc == 0:
                            fp = ffps.tile([128, N], F32, tag="fp", bufs=2)
                            for kc in range(NCH):
                                for mv in range(2):
                                    nc.tensor.matmul(
                                        fp[:, mv * 512:(mv + 1) * 512],
                                        lhsT=(w1_sb[:, kc, fc * 128:(fc + 1) * 128]),
                                        rhs=(mh_sb[img][:, kc, mv * 512:(mv + 1) * 512]),
                                        start=(kc == 0), stop=(kc == NCH - 1))
                            # PReLU(y) via ACT Prelu (alpha = slope; Lrelu ignores alpha on this HW)
                            nc.scalar.activation(
                                ffs[img][fc], fp,
                                mybir.ActivationFunctionType.Prelu,
                                bias=b1_sb[:, fc:fc + 1], alpha=a_slope)
                        for mv in range(2):
                            nc.tensor.matmul(
                                outp[:, mv * 512:(mv + 1) * 512],
                                lhsT=(w2_sb[:, fc, mc * 128:(mc + 1) * 128]),
                                rhs=(ffs[img][fc][:, mv * 512:(mv + 1) * 512]),
                                start=(fc == 0), stop=(fc == NFC - 1))
                    nc.vector.tensor_add(u_sb[img][:, mc, :], outp,
                                         mh_sb[img][:, mc, :])
                    for sg in range(2):
                        nc.vector.bn_stats(
                            out=st2[:, mc, img * 2 + sg, :],
                            in_=u_sb[img][:, mc, sg * 512:(sg + 1) * 512])

        # =========== BN2 + output ===========
        s2_sb = work.tile([128, NCH], F32, name="s2_sb", tag="bns2")
        t2_sb = work.tile([128, NCH], F32, name="t2_sb", tag="bnt2")
        cc2 = _bn_allreduce(tc, nc, work, dram, "bn2", st2)
        _bn_finish(tc, nc, work, "bn2", cc_out=cc2, gam_sb=gam_sb,
                   bet_sb=bet_sb, scale_out=s2_sb, shift_out=t2_sb)
        for img in range(BL):
            outr = out_d.ap()[img].rearrange("(c p) n -> p c n", p=128)
            # ch0 apply on ACT, ch1 on DVE (o_sb is dead -> reuse as staging)
            nc.scalar.activation(o_sb[img][:, 0, :], u_sb[img][:, 0, :],
                                 mybir.ActivationFunctionType.Identity,
                                 bias=t2_sb[:, 0:1], scale=s2_sb[:, 0:1])
            nc.sync.dma_start(out=outr[:, 0, :], in_=o_sb[img][:, 0, :])
            nc.vector.tensor_scalar(
                out=o_sb[img][:, 1, :], in0=u_sb[img][:, 1, :],
                scalar1=s2_sb[:, 1:2], scalar2=t2_sb[:, 1:2],
                op0=mybir.AluOpType.mult, op1=mybir.AluOpType.add)
            nc.gpsimd.dma_start(out=outr[:, 1, :], in_=o_sb[img][:, 1, :])


def _bn_allreduce(tc, nc, work, dram, name, stats):
    """Aggregate all local bn_stats, pack [mean, msq], one 2KB AllReduce."""
    mv_t = work.tile([128, NCH, 2], F32, name=f"{name}_mv", tag=f"{name}_mv")
    pk = work.tile([128, NCH, 2], F32, name=f"{name}_pk", tag=f"{name}_pk")
    for ch in range(NCH):
        nc.vector.bn_aggr(out=mv_t[:, ch, :], in_=stats[:, ch, :, :])
        nc.vector.tensor_mul(pk[:, ch, 0:1], mv_t[:, ch, 0:1], mv_t[:, ch, 0:1])
        nc.vector.tensor_add(pk[:, ch, 1:2], mv_t[:, ch, 1:2], pk[:, ch, 0:1])
        nc.vector.tensor_copy(pk[:, ch, 0:1], mv_t[:, ch, 0:1])
    cc_in = dram.tile([128 * NCH * 2], F32, name=f"{name}_cc_in",
                      tag=f"{name}_cc_in")
    cc_out = dram.tile([128 * NCH * 2], F32, name=f"{name}_cc_out",
                       tag=f"{name}_cc_out", addr_space="Shared")
    nc.sync.dma_start(out=cc_in.rearrange("(p k) -> p k", p=128), in_=pk)
    nc.gpsimd.collective_compute(
        "AllReduce", mybir.AluOpType.add,
        replica_groups=[list(range(NCORES))],
        ins=[cc_in.opt()], outs=[cc_out.opt()])
    return cc_out


def _bn_finish(tc, nc, work, name, *, cc_out, gam_sb, bet_sb,
               scale_out, shift_out):
    """Turn the AllReduced [mean, msq] sums into per-channel scale/shift.
    rsqrt is DVE-only (bit-trick seed + Newton) to avoid an ACT table
    switch on the critical path."""
    sg_t = work.tile([128, NCH, 2], F32, name=f"{name}_sg", tag=f"{name}_sg")
    nc.sync.dma_start(out=sg_t, in_=cc_out.rearrange("(p k) -> p k", p=128))
    g8 = work.tile([128, NCH, 2], F32, name=f"{name}_g8", tag=f"{name}_g8")
    nc.vector.tensor_scalar_mul(g8, sg_t, 1.0 / NCORES)
    # var = msq - mean^2 + eps   (both channel chunks at once)
    var_t = work.tile([128, NCH], F32, name=f"{name}_var", tag=f"{name}_var")
    nc.vector.tensor_mul(var_t, g8[:, :, 0], g8[:, :, 0])
    nc.vector.tensor_sub(var_t, g8[:, :, 1], var_t)
    nc.vector.tensor_scalar_add(var_t, var_t, EPS)
    # rstd = rsqrt(var): bit-trick seed + 2 Newton iterations (~fp32)
    rs = work.tile([128, NCH], F32, name=f"{name}_rs", tag=f"{name}_rs")
    vi = var_t.bitcast(mybir.dt.int32)
    ri = rs.bitcast(mybir.dt.int32)
    nc.vector.tensor_scalar(out=ri, in0=vi, scalar1=1, scalar2=None,
                            op0=mybir.AluOpType.arith_shift_right)
    nc.vector.tensor_scalar(out=ri, in0=ri, scalar1=-1, scalar2=0x5f3759df,
                            op0=mybir.AluOpType.mult,
                            op1=mybir.AluOpType.add)
    half = work.tile([128, NCH], F32, name=f"{name}_half", tag=f"{name}_half")
    nc.vector.tensor_scalar_mul(half, var_t, -0.5)
    tmp = work.tile([128, NCH], F32, name=f"{name}_tmp", tag=f"{name}_tmp")
    for _ in range(2):
        nc.vector.tensor_mul(tmp, rs, rs)
        nc.vector.tensor_mul(tmp, tmp, half)
        nc.vector.tensor_scalar_add(tmp, tmp, 1.5)
        nc.vector.tensor_mul(rs, rs, tmp)
    nc.vector.tensor_mul(scale_out, gam_sb, rs)
    # shift = beta - mean * scale
    nc.vector.tensor_mul(tmp, g8[:, :, 0], scale_out)
    nc.vector.tensor_sub(shift_out, bet_sb, tmp)


_COMPILED = None


def _get_compiled(a_slope: float):
    global _COMPILED
    if _COMPILED is None or _COMPILED[0] != a_slope:
        _COMPILED = (a_slope, _build(a_slope))
    return _COMPILED[1]


def _prep_inputs(inputs):
    x = np.ascontiguousarray(np.asarray(inputs["x"], dtype=np.float32))
    Wq = np.asarray(inputs["Wq"], dtype=np.float32)
    Wk = np.asarray(inputs["Wk"], dtype=np.float32)
    Wv = np.asarray(inputs["Wv"], dtype=np.float32)
    bq = np.asarray(inputs["bq"], dtype=np.float32)
    W1 = np.asarray(inputs["W1"], dtype=np.float32)
    b1 = np.asarray(inputs["b1"], dtype=np.float32)
    W2 = np.asarray(inputs["W2"], dtype=np.float32)
    gamma = np.asarray(inputs["gamma"], dtype=np.float32)
    beta = np.asarray(inputs["beta"], dtype=np.float32)

    wqT = np.ascontiguousarray(Wq.reshape(C, C).T.astype(np.float16))
    wkT = np.ascontiguousarray(Wk.reshape(C, C).T.astype(np.float16))
    wvT = np.zeros((C, NH * DH), dtype=np.float16)
    for h in range(NH):
        wvT[:, h * DH:(h + 1) * DH] = Wv[h].T.astype(np.float16)
    common = {
        "wqT": wqT, "wkT": wkT, "wvT": wvT,
        "bq": np.ascontiguousarray(bq.reshape(C)),
        "w1T": np.ascontiguousarray(W1.T.astype(np.float16)),
        "b1s": np.ascontiguousarray(b1),
        "w2T": np.ascontiguousarray(W2.T.astype(np.float16)),
        "gamma": np.ascontiguousarray(gamma),
        "beta": np.ascontiguousarray(beta),
    }
    x16 = x.astype(np.float16)
    in_maps = []
    for c in range(NCORES):
        m = dict(common)
        m["x"] = np.ascontiguousarray(
            x16[c * BL:(c + 1) * BL].reshape(BL, C, N))
        in_maps.append(m)
    return in_maps


def kernel_ex(trace=False, **inputs):
    a_slope = float(np.asarray(inputs["a"]))
    nc = _get_compiled(a_slope)
    in_maps = _prep_inputs(inputs)
    res = bass_utils.run_bass_kernel_spmd(
        nc, in_maps, core_ids=list(range(NCORES)), trace=trace)
    out = np.empty((B, C, N), dtype=np.float32)
    for c in range(NCORES):
        out[c * BL:(c + 1) * BL] = res.results[c]["out"]
    return out.reshape(B, C, HH, WW), res


def kernel(**inputs):
    out, _ = kernel_ex(False, **inputs)
    return out
